# revision 67
# baseline (speedup 1.0000x reference)
"""DiT block kernel for Trainium2, SPMD data-parallel over batch across 8 NeuronCores.

Per-core computation (one batch element, N=1024 tokens, D=1024):
  adaLN1 -> qkv -> attention(16 heads, hd=64) -> proj + residual
  adaLN2 -> fc1 -> gelu(exact/erf) -> fc2 + residual

Layout strategy:
  - residual stream x kept token-major (tm) [tok_p, feat] fp32 in SBUF
  - LN stats via bn_stats along free dim; adaLN scale/shift broadcast across
    partitions via tiny PE ones-matmuls
  - normalized h transposed to feature-major (fm) bf16 via PE transpose
  - all big matmuls in bf16 (fp32 PSUM accumulation)
  - Q^T,K^T produced fm (weights stationary), V token-major (acts stationary)
    with a ones-column appended so the attention AV matmul also produces the
    softmax denominators (softmax computed without max subtraction - safe at
    these scales, exact same math as reference)
  - proj/fc2 act-stationary -> token-major outputs + fp32 residual adds
"""

import sys

if "/opt/trn_rl_repo" not in sys.path:
    sys.path.insert(0, "/opt/trn_rl_repo")

from contextlib import ExitStack

import ml_dtypes
import numpy as np

import concourse.bacc as bacc
import concourse.bass as bass
import concourse.mybir as mybir
import concourse.tile as tile
from concourse.bass import ds, ts
from concourse.masks import make_identity

FP32 = mybir.dt.float32
BF16 = mybir.dt.bfloat16
AF = mybir.ActivationFunctionType
ALU = mybir.AluOpType

B, N, D = 8, 1024, 1024
H, HD, DFF = 16, 64, 4096
P = 128
NT = N // P   # 8 token tiles
KT = D // P   # 8 feature k-tiles
EPS = 1e-6
# "erf": exact gelu via Erf activation (not implemented in CoreSim, HW ok)
# "tanh": tanh-approx gelu from Square+Tanh (CoreSim-compatible fallback)
GELU_MODE = "erf"

BF16_NP = ml_dtypes.bfloat16


def build():
    """Build the single-core program (same program on all 8 cores)."""
    nc = bacc.Bacc(None, target_bir_lowering=False, debug=False)
    names = {}

    with tile.TileContext(nc) as tc:
        with ExitStack() as root:
            dram = root.enter_context(tc.tile_pool(name="dram", bufs=1, space="DRAM"))

            def din(nm, shape, dt=BF16):
                t = dram.tile(shape, dt, kind="ExternalInput", name=nm)
                names[nm] = t.name
                return t

            x_d = din("x", [N, D])  # bf16 (residual re-materialized in fp32)
            condt_d = din("condt", [P, KT])
            wqk_d = din("wqk", [KT, 16, P, P])
            wv_d = din("wv", [KT, 2, P, 512])
            wproj_d = din("wproj", [KT, 2, P, 512])
            wada1_d = din("wada1", [KT, 4, P, 512])
            wada2_d = din("wada2", [KT, 4, P, 512])
            wfc1_d = din("wfc1", [KT, 32, P, P])
            wfc2_d = din("wfc2", [32, 2, P, 512])
            bada1_d = din("bada1", [1, 2 * D], FP32)
            bada2_d = din("bada2", [1, 2 * D], FP32)
            bqt_d = din("bqt", [P, KT], FP32)
            bkt_d = din("bkt", [P, KT], FP32)
            bvt_d = din("bvt", [P, KT], FP32)
            bfc1t_d = din("bfc1t", [P, 32], FP32)
            bfc1ts_d = din("bfc1ts", [P, 32], FP32)
            bproj_d = din("bprojbf", [1, D])
            bfc2_d = din("bfc2bf", [1, D])
            out_d = dram.tile([N, D], FP32, kind="ExternalOutput", name="out")
            names["out"] = out_d.name

            # ---------------- constants / small inputs ----------------
            const = root.enter_context(tc.tile_pool(name="const", bufs=1))
            psum = root.enter_context(tc.tile_pool(name="psum", bufs=6, space="PSUM"))

            def pt(nm="ps"):
                return psum.tile([P, 512], FP32, tag="ps", name=nm, bufs=4)

            def pav(nm="pav"):
                return psum.tile([P, 512], FP32, tag="pav", name=nm, bufs=2)

            def pt_tr(nm="pstr"):
                return psum.tile([P, P], BF16, tag="pstr", name=nm, bufs=2)

            def pt_b(nm="psb"):
                # broadcast psum; shares banks with the transpose tag (the two
                # are used in disjoint phases)
                return psum.tile([P, 512], FP32, tag="pstr", name=nm, bufs=2)

            condt_sb = const.tile([P, KT], BF16, name="condt_sb")
            nc.sync.dma_start(out=condt_sb[:, :], in_=condt_d[:, :])
            bqt_sb = const.tile([P, KT], FP32, name="bqt_sb")
            nc.sync.dma_start(out=bqt_sb[:, :], in_=bqt_d[:, :])
            bkt_sb = const.tile([P, KT], FP32, name="bkt_sb")
            nc.sync.dma_start(out=bkt_sb[:, :], in_=bkt_d[:, :])
            bvt_sb = const.tile([P, KT], FP32, name="bvt_sb")
            nc.sync.dma_start(out=bvt_sb[:, :], in_=bvt_d[:, :])
            bfc1t_sb = const.tile([P, 32], FP32, name="bfc1t_sb")
            nc.sync.dma_start(out=bfc1t_sb[:, :], in_=bfc1t_d[:, :])
            bfc1ts_sb = const.tile([P, 32], FP32, name="bfc1ts_sb")
            nc.sync.dma_start(out=bfc1ts_sb[:, :], in_=bfc1ts_d[:, :])
            bproj_sb = const.tile([1, D], BF16, name="bproj_sb")
            nc.sync.dma_start(out=bproj_sb[:, :], in_=bproj_d[:, :])
            bfc2_sb = const.tile([1, D], BF16, name="bfc2_sb")
            nc.sync.dma_start(out=bfc2_sb[:, :], in_=bfc2_d[:, :])
            bada_d = (bada1_d, bada2_d)

            ones_bf = const.tile([1, P], BF16, name="ones_bf")
            nc.vector.memset(ones_bf[:, :], 1.0)
            ident_bf = const.tile([P, P], BF16, name="ident_bf")
            make_identity(nc, ident_bf[:, :])
            zero_col = const.tile([P, 1], FP32, name="zero_col")
            nc.vector.memset(zero_col[:, :], 0.0)
            nc.const_aps.aps[(FP32, 0.0)] = zero_col[:, :]
            eps_col = const.tile([P, 1], FP32, name="eps_col")
            nc.vector.memset(eps_col[:, :], EPS)

            def act_recip(out, in_):
                # ACT-table reciprocal (~1e-4 rel err, fine for softmax
                # denominators / rstd); DVE InstReciprocal costs ~3.3us/call
                ins_ = [nc.scalar.lower_ap(in_)]
                for val in (0.0, 1.0, 0.0):  # bias, scale, alpha
                    ins_.append(mybir.ImmediateValue(dtype=FP32, value=val))
                return nc.scalar.add_instruction(
                    mybir.InstActivation(
                        name=nc.scalar.bass.get_next_instruction_name(),
                        func=AF.Reciprocal,
                        ins=ins_,
                        outs=[nc.scalar.lower_ap(out)],
                    )
                )

            # warm the PE clock (HAM) with dep-free matmuls while the first
            # weight/x DMAs stream in
            warm_src = const.tile([P, 512], BF16, name="warm_src")
            nc.vector.memset(warm_src[:, :], 0.0)
            warm_sink = const.tile([1, 8], FP32, name="warm_sink")
            wps = pt("warm")
            for _ in range(24):
                nc.tensor.matmul(
                    wps[:, :], lhsT=ident_bf[:, :], rhs=warm_src[:, :],
                    start=True, stop=True,
                )
            nc.vector.tensor_copy(out=warm_sink[:, :], in_=wps[0:1, 0:8])

            # ---------------- adaLN scale/shift rows ----------------
            ss_bf = []  # per ada: ((1+scale) bf16 row, shift bf16 row)

            def ada_block(ai, wada_d, push_cb=None):
                s_bf = const.tile([1, D], BF16, name=f"sbf{ai}")
                sh_bf = const.tile([1, D], BF16, name=f"shbf{ai}")
                with tc.tile_pool(name=f"wada{ai}", bufs=8) as wada_pool, \
                     tc.tile_pool(name=f"adascr{ai}", bufs=2) as adascr:
                    for og in range(4):
                        bada = adascr.tile(
                            [1, 512], FP32, tag="bada", name="badat"
                        )
                        nc.sync.dma_start(
                            out=bada[:, :], in_=bada_d[ai][0:1, ds(og * 512, 512)]
                        )
                        ps = pt()
                        for kt in range(KT):
                            wt = wada_pool.tile(
                                [P, 512], BF16, tag="wada", name="wadat"
                            )
                            nc.sync.dma_start(out=wt[:, :], in_=wada_d[kt, og])
                            nc.tensor.matmul(
                                ps[0:1, :],
                                lhsT=condt_sb[:, kt : kt + 1],
                                rhs=wt[:, :],
                                start=(kt == 0),
                                stop=(kt == KT - 1),
                            )
                        t = adascr.tile([1, 512], FP32, tag="sst", name="sst")
                        nc.vector.tensor_tensor(
                            out=t[:, :], in0=ps[0:1, :], in1=bada[:, :], op=ALU.add
                        )
                        if og < 2:
                            nc.vector.tensor_scalar(
                                out=s_bf[:, ds(og * 512, 512)], in0=t[:, :],
                                scalar1=1.0, scalar2=None, op0=ALU.add,
                            )
                        else:
                            nc.vector.tensor_copy(
                                out=sh_bf[:, ds((og - 2) * 512, 512)], in_=t[:, :]
                            )
                        if push_cb is not None:
                            push_cb(1)
                    ss_bf.append((s_bf, sh_bf))

            # load x first so LN1 stats can start as early as possible
            es_x = ExitStack()
            p_x = es_x.enter_context(tc.tile_pool(name="p_x", bufs=1))
            x_sb = p_x.tile([P, NT, D], BF16, name="x_sb")
            for tt in range(NT):
                nc.sync.dma_start(out=x_sb[:, tt, :], in_=x_d[ts(tt, P), :])

            ada_block(0, wada1_d)

            def bcast_row(row_bf, nm, pool):
                """[1, D] bf16 row -> [128, D] bf16 (PE ones-matmul broadcast)."""
                outt = pool.tile([P, D], BF16, name=nm)
                for og in range(2):
                    ps = pt()
                    nc.tensor.matmul(
                        ps[:, :],
                        lhsT=ones_bf[0:1, :],
                        rhs=row_bf[0:1, ds(og * 512, 512)],
                        start=True,
                        stop=True,
                    )
                    nc.vector.tensor_copy(out=outt[:, ds(og * 512, 512)], in_=ps[:, :])
                return outt

            def ln_tile(xt, s_b, sh_b, scr):
                """DVE chain: LayerNorm(xt)*(s_b) + sh_b -> bf16 tile."""
                st = scr.tile([P, 2, 6], FP32, tag="bnst", name="bnst")
                xr = xt.rearrange("p (s f) -> p s f", f=512)
                for sg in range(2):
                    nc.vector.bn_stats(out=st[:, sg, :], in_=xr[:, sg, :])
                mv = scr.tile([P, 2], FP32, tag="bnmv", name="bnmv")
                nc.vector.bn_aggr(out=mv[:, :], in_=st[:, :, :])
                nc.scalar.activation(
                    out=mv[:, 1:2], in_=mv[:, 1:2], func=AF.Sqrt,
                    bias=eps_col[:, 0:1],
                )
                nc.vector.reciprocal(out=mv[:, 1:2], in_=mv[:, 1:2])
                xn = scr.tile([P, D], FP32, tag="xn", name="xn", bufs=1)
                nc.vector.tensor_scalar(
                    out=xn[:, :], in0=xt, scalar1=mv[:, 0:1], scalar2=mv[:, 1:2],
                    op0=ALU.subtract, op1=ALU.mult,
                )
                nc.vector.tensor_tensor(
                    out=xn[:, :], in0=xn[:, :], in1=s_b[:, :], op=ALU.mult
                )
                hbf = scr.tile([P, D], BF16, tag="hbf", name="hbf", bufs=3)
                nc.vector.tensor_tensor(
                    out=hbf[:, :], in0=xn[:, :], in1=sh_b[:, :], op=ALU.add
                )
                return hbf

            def tr_tile(hbf, tt, hT):
                """PE-transpose a [128, D] bf16 tile into fm hT.
                (dma_start_transpose SBUF->SBUF hard-hangs the device)"""
                for ft in range(KT):
                    ps = pt_tr()
                    nc.tensor.transpose(ps[:, :], hbf[:, ts(ft, P)], ident_bf[:, :])
                    nc.vector.tensor_copy(out=hT[:, ft, ts(tt, P)], in_=ps[:, :])

            def ln_transpose(x_t, s_b, sh_b, hT, scr):
                for tt in range(NT):
                    hbf = ln_tile(x_t[:, tt, :], s_b, sh_b, scr)
                    tr_tile(hbf, tt, hT)

            # ---------------- phase B: LN1 + transpose ----------------
            es_h1 = ExitStack()
            p_h1 = es_h1.enter_context(tc.tile_pool(name="p_h1", bufs=1))
            h1T = p_h1.tile([P, KT, N], BF16, name="h1T")
            with tc.tile_pool(name="lnscr1", bufs=2) as scr1, \
                 tc.tile_pool(name="sb1", bufs=1) as sb1:
                s1b = bcast_row(ss_bf[0][0], "s1b", sb1)
                sh1b = bcast_row(ss_bf[0][1], "sh1b", sb1)
                ln_transpose(x_sb, s1b, sh1b, h1T, scr1)

            # ---------------- phase C: QKV ----------------
            es_qkv = ExitStack()
            p_qkv = es_qkv.enter_context(tc.tile_pool(name="p_qkv", bufs=1))
            qT = p_qkv.tile([P, KT, N], BF16, name="qT")
            # kTz[:, s, hf, :]: K^T of head 2*hf+s zero-padded to 128 partitions
            # (other head's rows zeroed) so S^T matmuls stream a full-width rhs
            kTz = p_qkv.tile([P, 2, KT, N], BF16, name="kTz")
            nc.gpsimd.memset(kTz[64:P, 0, :, :], 0.0)
            nc.gpsimd.memset(kTz[0:64, 1, :, :], 0.0)
            HDP = 72  # per-head V stride: 64 values + ones col + pad (16B aligned)
            V1 = p_qkv.tile([P, NT, H, HDP], BF16, name="V1")
            nc.gpsimd.memset(V1[:, :, :, HD:HDP], 0.0)
            nc.gpsimd.memset(V1[:, :, :, HD : HD + 1], 1.0)

            es_ctx = ExitStack()
            p_ctx = es_ctx.enter_context(
                tc.tile_pool(name="p_ctx", bufs=1, side="right")
            )
            ctxT = p_ctx.tile([P, KT, N], BF16, name="ctxT")

            # attention units interleave with the QKV emission so the PE has
            # matmul work while the ACT engine grinds through softmax exps
            units = [
                (2 * hf + s, qg)
                for hf in range(KT) for s in range(2) for qg in range(2)
            ]
            AV_LAG = 2    # units of S/exp emitted ahead of each AV
            TAIL_LAG = 3  # units of S emitted ahead of each normalization tail

            with tc.tile_pool(name="wqk", bufs=3) as wqk_pool, \
                 tc.tile_pool(name="wv", bufs=2) as wv_pool, \
                 tc.tile_pool(name="etp", bufs=3) as et_pool, \
                 tc.tile_pool(name="ascr", bufs=2) as ascr:

                def qk(oft):
                    wt = wqk_pool.tile([P, KT, P], BF16, tag="wqk", name="wqkt")
                    for kt in range(KT):
                        nc.sync.dma_start(out=wt[:, kt, :], in_=wqk_d[kt, oft])
                    for tg in range(2):
                        ps = pt()
                        for kt in range(KT):
                            nc.tensor.matmul(
                                ps[:, :],
                                lhsT=wt[:, kt, :],
                                rhs=h1T[:, kt, ds(tg * 512, 512)],
                                start=(kt == 0),
                                stop=(kt == KT - 1),
                            )
                        if oft < 8:
                            nc.vector.tensor_scalar(
                                out=qT[:, oft, ds(tg * 512, 512)], in0=ps[:, :],
                                scalar1=bqt_sb[:, oft : oft + 1], scalar2=None,
                                op0=ALU.add,
                            )
                        else:
                            hf = oft - 8
                            nc.vector.tensor_scalar(
                                out=kTz[0:64, 0, hf, ds(tg * 512, 512)],
                                in0=ps[0:64, :],
                                scalar1=bkt_sb[0:64, hf : hf + 1], scalar2=None,
                                op0=ALU.add,
                            )
                            nc.vector.tensor_scalar(
                                out=kTz[64:P, 1, hf, ds(tg * 512, 512)],
                                in0=ps[64:P, :],
                                scalar1=bkt_sb[64:P, hf : hf + 1], scalar2=None,
                                op0=ALU.add,
                            )

                def vblock(og, push_every=0):
                    wvt = wv_pool.tile([P, KT, 512], BF16, tag="wv", name="wvt")
                    for kt in range(KT):
                        nc.sync.dma_start(out=wvt[:, kt, :], in_=wv_d[kt, og])
                    for tt in range(NT):
                        ps = pt()
                        for kt in range(KT):
                            nc.tensor.matmul(
                                ps[:, :],
                                lhsT=h1T[:, kt, ts(tt, P)],
                                rhs=wvt[:, kt, :],
                                start=(kt == 0),
                                stop=(kt == KT - 1),
                            )
                        nc.vector.tensor_copy(
                            out=V1[:, tt, ds(og * 8, 8), 0:HD],
                            in_=ps[:, :].rearrange("p (h e) -> p h e", e=HD),
                        )
                        if push_every and tt % push_every == push_every - 1:
                            push(1)

                def emit_S(h, qg, et):
                    hf = h // 2
                    for kt in range(KT):
                        ps = pt()
                        nc.tensor.matmul(
                            ps[:, :],
                            lhsT=kTz[:, h % 2, hf, ts(kt, P)],
                            rhs=qT[:, hf, ds(qg * 512, 512)],
                            start=True,
                            stop=True,
                        )
                        nc.scalar.activation(
                            out=et[:, kt, :], in_=ps[:, :], func=AF.Exp,
                            scale=float(HD) ** -0.5,
                        )

                def emit_AV(h, qg, et):
                    psc = pav()
                    for kt in range(KT):
                        nc.tensor.matmul(
                            psc[0:HDP, :],
                            lhsT=V1[:, kt, h, :],
                            rhs=et[:, kt, :],
                            start=(kt == 0),
                            stop=(kt == KT - 1),
                        )
                    # DVE reciprocal is slow (~3.3us, single lane) but off the
                    # critical path via TAIL_LAG; ACT reciprocal would thrash
                    # the activation table against the softmax Exp (1.3us/swap)
                    rrow = ascr.tile([1, 512], FP32, tag="rrow", name="rrow")
                    nc.vector.reciprocal(out=rrow[:, :], in_=psc[HD : HD + 1, :])
                    rbf = ascr.tile([1, 512], BF16, tag="rbf", name="rbf")
                    nc.vector.tensor_copy(out=rbf[:, :], in_=rrow[:, :])
                    return psc, rbf

                def emit_tail(h, qg, psc, rbf):
                    m0 = 64 * (h % 2)
                    hf = h // 2
                    psb = pt_b()
                    nc.tensor.matmul(
                        psb[0:HD, :],
                        lhsT=ones_bf[0:1, 0:HD],
                        rhs=rbf[0:1, :],
                        start=True,
                        stop=True,
                    )
                    rb = ascr.tile([HD, 512], FP32, tag="rb", name="rb")
                    nc.vector.tensor_copy(out=rb[:, :], in_=psb[0:HD, :])
                    ctmp = ascr.tile([HD, 512], FP32, tag="ctmp", name="ctmp")
                    nc.vector.tensor_tensor(
                        out=ctmp[:, :], in0=psc[0:HD, :], in1=rb[:, :],
                        op=ALU.mult,
                    )
                    if m0 == 0:
                        nc.vector.tensor_scalar(
                            out=ctxT[0:HD, hf, ds(qg * 512, 512)], in0=ctmp[:, :],
                            scalar1=bvt_sb[0:HD, hf : hf + 1], scalar2=None,
                            op0=ALU.add,
                        )
                    else:
                        # DVE cannot shift partitions; stage at base 0 then
                        # DMA-shift to partitions 64..127
                        cstg = ascr.tile([HD, 512], BF16, tag="cstg", name="cstg")
                        nc.vector.tensor_scalar(
                            out=cstg[:, :], in0=ctmp[:, :],
                            scalar1=bvt_sb[m0 : m0 + HD, hf : hf + 1],
                            scalar2=None, op0=ALU.add,
                        )
                        nc.sync.dma_start(
                            out=ctxT[m0 : m0 + HD, hf, ds(qg * 512, 512)],
                            in_=cstg[:, :],
                        )

                ets, avs = {}, {}
                pipe = {"i": 0}

                def push(n):
                    for _ in range(n):
                        i = pipe["i"]
                        if i >= len(units) + TAIL_LAG:
                            return
                        if i < len(units):
                            ets[i] = et_pool.tile(
                                [P, KT, 512], BF16, tag="et", name="et"
                            )
                            emit_S(*units[i], ets[i])
                        j = i - AV_LAG
                        if 0 <= j < len(units):
                            avs[j] = emit_AV(*units[j], ets.pop(j))
                        k = i - TAIL_LAG
                        if k >= 0:
                            emit_tail(*units[k], *avs.pop(k))
                        pipe["i"] += 1

                # schedule: Q+K for head-pairs 0-3, V block 0, then interleave
                # the remaining QKV emission with ready attention units
                for hf in range(4):
                    qk(hf)
                    qk(8 + hf)
                vblock(0)
                # cap pushes so no og1-head AV (unit >= 16) is emitted before
                # vblock(1) has written all of V1's second half
                ada_block(1, wada2_d, push_cb=push)
                for hf in range(4, 8):
                    qk(hf)
                    push(2)
                    qk(8 + hf)
                    push(1)
                vblock(1, push_every=4)
                push(len(units) + TAIL_LAG - pipe["i"])

            es_qkv.close()  # qT, kTz, V1 dead
            es_h1.close()  # h1T dead

            # ------- phase E+F: proj + residual + LN2 + transpose, fused -------
            # per token tile: proj MMs -> x1 row -> LN2 DVE chain; transposes
            # lag one tile so the PE never waits on the LN2 DVE chain
            es_x1 = ExitStack()
            p_x1 = es_x1.enter_context(
                tc.tile_pool(name="p_x1", bufs=1, side="right")
            )
            x1_sb = p_x1.tile([P, NT, D], FP32, name="x1_sb")
            es_f = ExitStack()
            p_f = es_f.enter_context(tc.tile_pool(name="p_f", bufs=1, side="right"))
            fT = p_f.tile([P, 32, N], BF16, name="fT")
            es_h2 = ExitStack()
            p_h2 = es_h2.enter_context(
                tc.tile_pool(name="p_h2", bufs=1, side="right")
            )
            h2T = p_h2.tile([P, KT, N], BF16, name="h2T")

            with tc.tile_pool(name="wp", bufs=2) as wp_pool, \
                 tc.tile_pool(name="sbE", bufs=1) as sbE, \
                 tc.tile_pool(name="lnscr2", bufs=2) as scr2:
                s2b = bcast_row(ss_bf[1][0], "s2b", sbE)
                sh2b = bcast_row(ss_bf[1][1], "sh2b", sbE)
                wpts = []
                for og in range(2):
                    wpt = wp_pool.tile([P, KT, 512], BF16, tag="wp", name="wpt")
                    for kt in range(KT):
                        nc.sync.dma_start(out=wpt[:, kt, :], in_=wproj_d[kt, og])
                    wpts.append(wpt)
                prev = None
                for tt in range(NT):
                    for og in range(2):
                        ps = pt()
                        for kt in range(KT):
                            nc.tensor.matmul(
                                ps[:, :],
                                lhsT=ctxT[:, kt, ts(tt, P)],
                                rhs=wpts[og][:, kt, :],
                                start=(kt == 0),
                                stop=False,
                            )
                        # bias as a K=1 accumulating matmul (frees a DVE pass)
                        nc.tensor.matmul(
                            ps[:, :],
                            lhsT=ones_bf[0:1, :],
                            rhs=bproj_sb[0:1, ds(og * 512, 512)],
                            start=False,
                            stop=True,
                        )
                        nc.vector.tensor_tensor(
                            out=x1_sb[:, tt, ds(og * 512, 512)], in0=ps[:, :],
                            in1=x_sb[:, tt, ds(og * 512, 512)], op=ALU.add,
                        )
                    hbf = ln_tile(x1_sb[:, tt, :], s2b, sh2b, scr2)
                    if prev is not None:
                        tr_tile(prev[0], prev[1], h2T)
                    prev = (hbf, tt)
                tr_tile(prev[0], prev[1], h2T)

            es_x.close()  # x dead

            # ---------------- phase G: fc1 + gelu ----------------
            es_w2 = ExitStack()
            w2_pool = es_w2.enter_context(tc.tile_pool(name="w2", bufs=2))
            with tc.tile_pool(name="w1", bufs=3) as w1_pool, \
                 tc.tile_pool(name="gscr", bufs=2) as gscr:
                for oft in range(32):
                    w1t = w1_pool.tile([P, KT, P], BF16, tag="w1", name="w1t")
                    for kt in range(KT):
                        nc.sync.dma_start(out=w1t[:, kt, :], in_=wfc1_d[kt, oft])
                    for tg in range(2):
                        ps = pt()
                        for kt in range(KT):
                            nc.tensor.matmul(
                                ps[:, :],
                                lhsT=w1t[:, kt, :],
                                rhs=h2T[:, kt, ds(tg * 512, 512)],
                                start=(kt == 0),
                                stop=(kt == KT - 1),
                            )
                        # u = psum + b ; f = (1+approx(u))*u
                        # (the 0.5 of exact gelu is folded into w_fc2)
                        u = gscr.tile([P, 512], FP32, tag="u", name="u")
                        nc.scalar.activation(
                            out=u[:, :], in_=ps[:, :], func=AF.Identity,
                            bias=bfc1t_sb[:, oft : oft + 1],
                        )
                        v = gscr.tile([P, 512], FP32, tag="v", name="v")
                        if GELU_MODE == "erf":
                            # v = erf(u / sqrt(2))
                            nc.scalar.activation(
                                out=v[:, :], in_=ps[:, :], func=AF.Erf,
                                scale=0.7071067811865476,
                                bias=bfc1ts_sb[:, oft : oft + 1],
                            )
                        else:
                            # v = tanh(sqrt(2/pi) * (u + 0.044715 u^3))
                            s = gscr.tile([P, 512], FP32, tag="s", name="s")
                            nc.scalar.activation(
                                out=s[:, :], in_=u[:, :], func=AF.Square
                            )
                            w_ = gscr.tile([P, 512], FP32, tag="w_", name="w_")
                            nc.vector.tensor_scalar(
                                out=w_[:, :], in0=s[:, :],
                                scalar1=0.044715 * 0.7978845608028654,
                                scalar2=0.7978845608028654,
                                op0=ALU.mult, op1=ALU.add,
                            )
                            z = gscr.tile([P, 512], FP32, tag="z", name="z")
                            nc.vector.tensor_tensor(
                                out=z[:, :], in0=w_[:, :], in1=u[:, :], op=ALU.mult
                            )
                            nc.scalar.activation(
                                out=v[:, :], in_=z[:, :], func=AF.Tanh
                            )
                        nc.vector.scalar_tensor_tensor(
                            out=fT[:, oft, ds(tg * 512, 512)], in0=v[:, :],
                            scalar=1.0, in1=u[:, :],
                            op0=ALU.add, op1=ALU.mult,
                        )

            es_h2.close()

            # ---------------- phase H: fc2 + residual ----------------
            # fc2 weights streamed in 4 of-chunks of 256 (w2 pool opened
            # before phase G so chunk DMAs prefetch under fc1 compute)
            with tc.tile_pool(name="hscr", bufs=3) as hscr:
                for oc in range(4):
                    w2t = w2_pool.tile([P, 32, 256], BF16, tag="w2", name="w2t")
                    for kt in range(32):
                        nc.sync.dma_start(
                            out=w2t[:, kt, :],
                            in_=wfc2_d[kt, oc // 2][:, ds((oc % 2) * 256, 256)],
                        )
                    for tt in range(NT):
                        ps = pt()
                        for kt in range(32):
                            nc.tensor.matmul(
                                ps[:, 0:256],
                                lhsT=fT[:, kt, ts(tt, P)],
                                rhs=w2t[:, kt, :],
                                start=(kt == 0),
                                stop=False,
                            )
                        nc.tensor.matmul(
                            ps[:, 0:256],
                            lhsT=ones_bf[0:1, :],
                            rhs=bfc2_sb[0:1, ds(oc * 256, 256)],
                            start=False,
                            stop=True,
                        )
                        ot = hscr.tile([P, 256], FP32, tag="ot", name="ot")
                        nc.vector.tensor_tensor(
                            out=ot[:, :], in0=ps[:, 0:256],
                            in1=x1_sb[:, tt, ds(oc * 256, 256)], op=ALU.add,
                        )
                        nc.sync.dma_start(
                            out=out_d[ts(tt, P), ds(oc * 256, 256)], in_=ot[:, :]
                        )

            es_w2.close()
            es_f.close()
            es_x1.close()
            es_ctx.close()

    nc.compile()
    return nc, names


def _bf(a):
    return np.ascontiguousarray(np.asarray(a, dtype=np.float32)).astype(BF16_NP)


def _f32(a):
    return np.ascontiguousarray(np.asarray(a, dtype=np.float32))


def prep_shared(w):
    """Host-side weight retiling (shared across cores)."""
    wqkv = np.asarray(w["w_qkv"], np.float32)
    shared = {
        "wqk": _bf(wqkv[:, : 2 * D].reshape(KT, P, 16, P).transpose(0, 2, 1, 3)),
        "wv": _bf(wqkv[:, 2 * D :].reshape(KT, P, 2, 512).transpose(0, 2, 1, 3)),
        "wproj": _bf(
            np.asarray(w["w_proj"], np.float32)
            .reshape(KT, P, 2, 512).transpose(0, 2, 1, 3)
        ),
        "wada1": _bf(
            np.asarray(w["w_ada1"], np.float32)
            .reshape(KT, P, 4, 512).transpose(0, 2, 1, 3)
        ),
        "wada2": _bf(
            np.asarray(w["w_ada2"], np.float32)
            .reshape(KT, P, 4, 512).transpose(0, 2, 1, 3)
        ),
        "wfc1": _bf(
            np.asarray(w["w_fc1"], np.float32)
            .reshape(KT, P, 32, P).transpose(0, 2, 1, 3)
        ),
        "wfc2": _bf(
            (np.asarray(w["w_fc2"], np.float32) * 0.5)
            .reshape(32, P, 2, 512).transpose(0, 2, 1, 3)
        ),
        "bada1": _f32(w["b_ada1"]).reshape(1, 2 * D),
        "bada2": _f32(w["b_ada2"]).reshape(1, 2 * D),
        "bqt": _f32(np.asarray(w["b_qkv"], np.float32)[:D].reshape(KT, P).T),
        "bkt": _f32(np.asarray(w["b_qkv"], np.float32)[D : 2 * D].reshape(KT, P).T),
        "bvt": _f32(np.asarray(w["b_qkv"], np.float32)[2 * D :].reshape(KT, P).T),
        "bfc1t": _f32(np.asarray(w["b_fc1"], np.float32).reshape(32, P).T),
        "bprojbf": _bf(w["b_proj"]).reshape(1, D),
        "bfc2bf": _bf(w["b_fc2"]).reshape(1, D),
    }
    shared["bfc1ts"] = _f32(shared["bfc1t"] * 0.7071067811865476)
    return shared


def make_in_maps(inputs, names):
    x = np.asarray(inputs["x"], np.float32)
    cond = np.asarray(inputs["condition"], np.float32)
    shared = prep_shared(inputs)
    in_maps = []
    for b in range(B):
        m = {
            names["x"]: _bf(x[b]),
            names["condt"]: _bf(cond[b].reshape(KT, P).T),
        }
        for k, v in shared.items():
            m[names[k]] = v
        in_maps.append(m)
    return in_maps


_CACHE = {}


def kernel(**inputs) -> np.ndarray:
    if "nc" not in _CACHE:
        _CACHE["nc"], _CACHE["names"] = build()
    nc, names = _CACHE["nc"], _CACHE["names"]
    from concourse.bass_utils import run_bass_kernel_spmd

    in_maps = make_in_maps(inputs, names)
    res = run_bass_kernel_spmd(nc, in_maps, core_ids=list(range(B)))
    out = np.stack([np.asarray(res.results[b][names["out"]]) for b in range(B)])
    return out.astype(np.float32)


if __name__ == "__main__":
    nc, names = build()
    print("built ok:", len(names), "tensors")
